# revision 18
# baseline (speedup 1.0000x reference)
"""Distributed Trainium2 Bass kernel for causal GQA attention block (v2b).

Problem (hardcoded): x [4, 2048, 1024] f32; wq [1024, 1024]; wk/wv [1024, 256];
wo [1024, 1024]. 16 q-heads, 4 kv-heads, head_dim 64, rms-norm on q/k (no
weight), rope (base 10000), q gain 1.5, causal SDPA, out-proj.

Sharding over 8 cores: core i -> batch b = i//2, head-half p = i%2.

v2b restructure vs v1:
- Software-pipelined emission: k/v/q projection + rope are sliced into
  512-token units and draped into the attention j-loop so the PE never
  idles long (keeps the HAM clock warm) and attention starts early.
- v projected directly token-major (stationary xT chunk) - no transposes.
- Out-proj own-half reads oT directly (stationary); only the peer half
  crosses the wire, via a masked ReduceScatter (host supplies a per-core
  shard mask and a row-permuted wo = [own-head rows; peer-head rows]).
- Per-(j,half) collectives to halve the collective latency at each use.
- Softmax normalization reads the PV PSUM accumulator in place.
"""
import sys

sys.path.insert(0, "/opt/trn_rl_repo")

import numpy as np
import ml_dtypes

import concourse.bacc as bacc
import concourse.mybir as mybir
import concourse.tile as tile
from concourse.bass_utils import run_bass_kernel_spmd

F32 = mybir.dt.float32
BF16 = mybir.dt.bfloat16
AF = mybir.ActivationFunctionType

N = 2048          # tokens
C = 1024          # model dim
DQ = 512          # local q out-features (8 heads x 64)
DKV = 128         # local kv out-features (2 kv heads x 64)
D = 64            # head dim
NCC = C // 128    # 8 contraction chunks
NQT = 4           # q tiles of 512
NTC = N // 128    # 16 token chunks
QK_GAIN = 1.5
ROPE_BASE = 10000.0
EXP_SCALE = QK_GAIN / np.sqrt(D).item()  # folded gain * 1/sqrt(D) = 0.1875
EPS = float(np.finfo(np.float32).eps)
RG = [[0, 1], [2, 3], [4, 5], [6, 7]]


def _host_tables():
    inv_freq = (1.0 / (ROPE_BASE ** (np.arange(0, D, 2, dtype=np.float64) / D)))  # [32]
    t = np.arange(N, dtype=np.float64)
    ang = np.outer(inv_freq, t)  # [32, N]
    cos32 = np.cos(ang)
    sin32 = np.sin(ang)
    cosT = np.tile(cos32, (4, 1)).astype(np.float32)  # [128, N]
    sinTs = np.concatenate([-sin32, sin32, -sin32, sin32], axis=0).astype(np.float32)
    q = np.arange(128)
    trimask = (q[None, :] >= q[:, None]).astype(np.float32)  # keep q >= k
    ones33 = np.zeros((128, 33), np.float32)
    ones33[0:64, 0] = 1.0
    ones33[64:128, 32] = 1.0
    exp33 = np.zeros((33, 64), np.float32)
    exp33[0, :] = 1.0
    exp33[32, :] = 1.0
    bf = ml_dtypes.bfloat16
    return {
        "cosT": cosT.astype(bf),
        "sinTs": sinTs.astype(bf),
        "trimask": trimask.astype(bf),
        "ones33": ones33.astype(bf),
        "exp33": exp33.astype(bf),
    }


def build():
    nc = bacc.Bacc(None, target_bir_lowering=False, num_devices=8)

    x_ext = nc.declare_dram_parameter("x", [C, N], BF16, isOutput=False)  # host pre-transposed
    wq_ext = nc.declare_dram_parameter("wq", [C, DQ], BF16, isOutput=False)
    wk_ext = nc.declare_dram_parameter("wk", [C, DKV], BF16, isOutput=False)
    wv_ext = nc.declare_dram_parameter("wv", [C, DKV], BF16, isOutput=False)
    wo_ext = nc.declare_dram_parameter("wo", [C, DQ], BF16, isOutput=False)
    msk_ext = nc.declare_dram_parameter("msk", [128, 2], F32, isOutput=False)
    out_ext = nc.declare_dram_parameter("out", [N, DQ], F32, isOutput=True)

    tabs = _host_tables()
    cosT_d = nc.inline_tensor(tabs["cosT"], name="cosT_d")
    sinTs_d = nc.inline_tensor(tabs["sinTs"], name="sinTs_d")
    trimask_d = nc.inline_tensor(tabs["trimask"], name="trimask_d")
    ones33_d = nc.inline_tensor(tabs["ones33"], name="ones33_d")
    exp33_d = nc.inline_tensor(tabs["exp33"], name="exp33_d")

    with tile.TileContext(nc) as tc:
        with (
            tc.tile_pool(name="dram", bufs=1, space="DRAM") as dram,
            tc.tile_pool(name="persist", bufs=1) as ps,
        ):
            # ---- persistent SBUF tensors ----
            xT = ps.tile([128, NCC, N], BF16, name="xT")          # X^T chunks
            wq_sb = ps.tile([128, NCC, DQ], BF16, name="wq_sb")
            wk_sb = ps.tile([128, NCC, DKV], BF16, name="wk_sb")
            wv_sb = ps.tile([128, NCC, DKV], BF16, name="wv_sb")
            wo_sb = ps.tile([128, NCC, DQ], BF16, name="wo_sb")
            cosT = ps.tile([128, N], BF16, name="cosT")
            sinTs = ps.tile([128, N], BF16, name="sinTs")
            trimask = ps.tile([128, 128], BF16, name="trimask")
            ones33 = ps.tile([128, 33], BF16, name="ones33")
            exp33 = ps.tile([33, 64], BF16, name="exp33")
            eps_sb = ps.tile([128, 1], F32, name="eps_sb")
            msk2 = ps.tile([128, 2], F32, name="msk2")
            kT_raw = ps.tile([128, N], BF16, name="kT_raw")
            qT_raw = ps.tile([128, 4, N], BF16, name="qT_raw")
            kTdA = ps.tile([128, N], BF16, name="kTdA")           # kv head A dup'd
            kTdB = ps.tile([128, N], BF16, name="kTdB")
            v_sb = ps.tile([128, NTC, 130], BF16, name="v_sb")    # [V_A|1|V_B|1]
            oT = ps.tile([128, 4, N], BF16, name="oT")            # own O^T (normed)

            # ---- phase A: stage inputs (host supplies bf16, x pre-transposed) ----
            for cc in range(NCC):
                nc.sync.dma_start(
                    out=xT[:, cc, :],
                    in_=x_ext[cc * 128:(cc + 1) * 128, :],
                )
            nc.gpsimd.dma_start(out=wk_sb[:], in_=wk_ext.rearrange("(a p) j -> p a j", p=128))
            nc.gpsimd.dma_start(out=wv_sb[:], in_=wv_ext.rearrange("(a p) j -> p a j", p=128))
            nc.gpsimd.dma_start(out=wq_sb[:], in_=wq_ext.rearrange("(a p) j -> p a j", p=128))
            nc.gpsimd.dma_start(out=wo_sb[:], in_=wo_ext.rearrange("(a p) j -> p a j", p=128))
            nc.gpsimd.dma_start(out=cosT[:], in_=cosT_d[:])
            nc.gpsimd.dma_start(out=sinTs[:], in_=sinTs_d[:])
            nc.gpsimd.dma_start(out=trimask[:], in_=trimask_d[:])
            nc.gpsimd.dma_start(out=ones33[:], in_=ones33_d[:])
            nc.gpsimd.dma_start(out=exp33[:], in_=exp33_d[:])
            nc.gpsimd.dma_start(out=msk2[:], in_=msk_ext[:])
            nc.gpsimd.memset(eps_sb[:], EPS)
            nc.gpsimd.memset(v_sb[:, :, 64:65], 1.0)
            nc.gpsimd.memset(v_sb[:, :, 129:130], 1.0)

            with (
                tc.tile_pool(name="u_psum", bufs=1, space="PSUM") as up,
                tc.tile_pool(name="u_sbuf", bufs=3) as bs,
            ):
                # ---------- pipeline units ----------
                def kunit(qt):
                    pp = up.tile([128, 512], F32, tag="mm", bufs=2, name=f"kpp{qt}")
                    for cc in range(NCC):
                        nc.tensor.matmul(
                            pp[:], wk_sb[:, cc, :], xT[:, cc, qt * 512:(qt + 1) * 512],
                            start=(cc == 0), stop=(cc == NCC - 1),
                        )
                    nc.vector.tensor_copy(kT_raw[:, qt * 512:(qt + 1) * 512], pp[:])

                def vunit(tcx):
                    pv = up.tile([128, 128], F32, tag="mm", bufs=2, name=f"vpp{tcx}")
                    for cc in range(NCC):
                        nc.tensor.matmul(
                            pv[:], xT[:, cc, tcx * 128:(tcx + 1) * 128], wv_sb[:, cc, :],
                            start=(cc == 0), stop=(cc == NCC - 1),
                        )
                    nc.vector.tensor_copy(v_sb[:, tcx, 0:64], pv[:, 0:64])
                    nc.vector.tensor_copy(v_sb[:, tcx, 65:129], pv[:, 64:128])

                def qunit(jq, m):
                    pp = up.tile([128, 512], F32, tag="mm", bufs=2, name=f"qpp{jq}_{m}")
                    for cc in range(NCC):
                        nc.tensor.matmul(
                            pp[:], wq_sb[:, cc, m * 128:(m + 1) * 128],
                            xT[:, cc, jq * 512:(jq + 1) * 512],
                            start=(cc == 0), stop=(cc == NCC - 1),
                        )
                    nc.vector.tensor_copy(qT_raw[:, m, jq * 512:(jq + 1) * 512], pp[:])

                def rope_unit(src, dst, sl0):
                    sl = slice(sl0, sl0 + 512)
                    sq = bs.tile([128, 512], BF16, tag="sq", bufs=2, name="sq")
                    nc.vector.tensor_mul(sq[:], src[:, sl], src[:, sl])
                    msp = up.tile([33, 512], F32, tag="mm", bufs=2, name="msp")
                    nc.tensor.matmul(msp[:], ones33[:], sq[:], start=True, stop=True)
                    lnv = bs.tile([33, 512], F32, tag="lnv", bufs=2, name="lnv")
                    nc.scalar.activation(lnv[:], msp[:], AF.Ln, bias=eps_sb[0:33, :], scale=1.0 / D)
                    rr2 = bs.tile([33, 512], BF16, tag="rr2", bufs=2, name="rr2")
                    nc.scalar.activation(rr2[:], lnv[:], AF.Exp, scale=-0.5)
                    rot = bs.tile([128, 512], BF16, tag="rot", bufs=2, name="rot")
                    nc.vector.tensor_copy(rot[0:32], src[32:64, sl])
                    nc.vector.tensor_copy(rot[32:64], src[0:32, sl])
                    nc.vector.tensor_copy(rot[64:96], src[96:128, sl])
                    nc.vector.tensor_copy(rot[96:128], src[64:96, sl])
                    t1 = bs.tile([128, 512], BF16, tag="t1", bufs=2, name="t1")
                    nc.vector.tensor_mul(t1[:], src[:, sl], cosT[:, sl])
                    nc.vector.tensor_mul(rot[:], rot[:], sinTs[:, sl])
                    nc.vector.tensor_add(t1[:], t1[:], rot[:])
                    rbp = up.tile([128, 512], F32, tag="mm", bufs=2, name="rbp")
                    nc.tensor.matmul(rbp[0:64], exp33[0:1, :], rr2[0:1, :], start=True, stop=True)
                    nc.tensor.matmul(rbp[64:128], exp33[32:33, :], rr2[32:33, :], start=True, stop=True)
                    nc.vector.tensor_mul(dst[:, sl], t1[:], rbp[:])

                def krope(qt):
                    sl0 = qt * 512
                    sl = slice(sl0, sl0 + 512)
                    rope_unit(kT_raw, kTdA, sl0)
                    nc.vector.tensor_copy(kTdB[0:64, sl], kTdA[64:128, sl])
                    nc.vector.tensor_copy(kTdB[64:128, sl], kTdA[64:128, sl])
                    nc.vector.tensor_copy(kTdA[64:128, sl], kTdA[0:64, sl])

                def qrope(jq, m):
                    rope_unit(qT_raw[:, m, :], qT_raw[:, m, :], jq * 512)

                # ---------- collectives + out-proj ----------
                cc_outs = {}   # j -> cout dram tile
                og_tiles = {}  # j -> og sbuf tile [128, 4, 512]
                po_tiles = {}  # (j, tt) -> po psum tile
                bg = []        # background drape queue

                def emit_rs(j):
                    # one ReduceScatter per j-tile; the og LOAD is deferred to
                    # the point of use so its RS-wait never blocks the sync
                    # queue ahead of later DMAs.
                    cm = bs.tile([128, 2, 4, 512], BF16, tag="cm", bufs=2, name=f"cm{j}")
                    for s in range(2):
                        nc.vector.tensor_scalar_mul(
                            cm[:, s], oT[:, :, 512 * j:512 * (j + 1)],
                            msk2[:, s:s + 1],
                        )
                    cin = dram.tile([2, 128, 4, 512], BF16, name=f"cc_in{j}")
                    cout = dram.tile([128, 4, 512], BF16, name=f"cc_out{j}")
                    cc_outs[j] = cout
                    nc.sync.dma_start(out=cin.rearrange("s p m x -> p s m x"), in_=cm[:])
                    nc.gpsimd.collective_compute(
                        "ReduceScatter",
                        mybir.AluOpType.add,
                        replica_groups=RG,
                        ins=[cin.opt()],
                        outs=[cout.opt()],
                    )

                def load_og(jo):
                    og = bs.tile([128, 4, 512], BF16, tag="og", bufs=2, name=f"og{jo}")
                    og_tiles[jo] = og
                    nc.sync.dma_start(out=og[:], in_=cc_outs[jo][:])

                def emit_po_own(jo, tt):
                    tcix = jo * 4 + tt
                    po = up.tile([128, 512], F32, tag="o", bufs=2, name=f"po{jo}_{tt}")
                    po_tiles[(jo, tt)] = po
                    for rc in range(4):  # own half straight from oT
                        nc.tensor.matmul(
                            po[:], oT[:, rc, tcix * 128:(tcix + 1) * 128], wo_sb[:, rc, :],
                            start=(rc == 0), stop=False,
                        )

                def emit_po_peer(jo, tt):
                    tcix = jo * 4 + tt
                    po = po_tiles[(jo, tt)]
                    og = og_tiles[jo]
                    for rc in range(4):  # peer half from ReduceScatter output
                        nc.tensor.matmul(
                            po[:], og[:, rc, tt * 128:(tt + 1) * 128], wo_sb[:, 4 + rc, :],
                            start=False, stop=(rc == 3),
                        )
                    ev = bs.tile([128, 512], F32, tag="ev", bufs=2, name=f"ev{jo}_{tt}")
                    nc.vector.tensor_copy(ev[:], po[:])
                    nc.sync.dma_start(out=out_ext[tcix * 128:(tcix + 1) * 128, :], in_=ev[:])

                def emit_po(jo, tt):
                    emit_po_own(jo, tt)
                    emit_po_peer(jo, tt)

                # ---------- attention ----------
                def attention(j, m):
                    kT = kTdA if m < 2 else kTdB
                    vslot = 0 if m < 2 else 65
                    oab = up.tile([65, 2, 512], F32, tag="o", bufs=2, name=f"oab{j}_{m}")
                    nkc = 4 * (j + 1)

                    def emit_scores(kc):
                        i = kc - 4 * j
                        off = max(0, 128 * i)
                        w = 512 - off
                        q0 = 512 * j + off
                        sAB = up.tile([128, 2, 512], F32, tag="mm", bufs=2, name=f"sAB{kc}")
                        nc.tensor.matmul(
                            sAB[:, 0, 0:w], kT[0:64, kc * 128:(kc + 1) * 128],
                            qT_raw[0:64, m, q0:q0 + w], start=True, stop=True,
                            tile_position=(0, 0),
                        )
                        nc.tensor.matmul(
                            sAB[:, 1, 0:w], kT[64:128, kc * 128:(kc + 1) * 128],
                            qT_raw[64:128, m, q0:q0 + w], start=True, stop=True,
                            tile_position=(64, 0),
                        )
                        pAB = bs.tile([128, 2, 512], BF16, tag="pAB", bufs=5, name=f"pAB{kc}")
                        nc.scalar.activation(
                            pAB[:, :, 0:w], sAB[:, :, 0:w], AF.Exp, scale=EXP_SCALE
                        )
                        if i >= 0:
                            nc.vector.tensor_mul(
                                pAB[:, :, 0:128], pAB[:, :, 0:128],
                                trimask.rearrange("p (a b) -> p a b", a=1).broadcast_to([128, 2, 128]),
                            )
                        return pAB

                    def emit_pv(kc, pAB):
                        i = kc - 4 * j
                        off = max(0, 128 * i)
                        w = 512 - off
                        nc.tensor.matmul(
                            oab[:, 0, off:512], v_sb[:, kc, vslot:vslot + 65],
                            pAB[:, 0, 0:w], start=(kc == 0), stop=(kc == nkc - 1),
                            skip_group_check=True,
                        )
                        nc.tensor.matmul(
                            oab[:, 1, off:512], v_sb[:, kc, vslot:vslot + 65],
                            pAB[:, 1, 0:w], start=(kc == 0), stop=(kc == nkc - 1),
                            skip_group_check=True,
                        )

                    staged = []
                    for kc in range(nkc):
                        staged.append((kc, emit_scores(kc)))
                        if len(staged) == 2:
                            for kcx, px in staged:
                                emit_pv(kcx, px)
                            staged = []
                            # drape one background unit between kc-pairs so
                            # the PE fills Scalar-bound gaps without ever
                            # delaying the next scores by a large burst
                            if bg:
                                bg.pop(0)()
                    for kcx, px in staged:
                        emit_pv(kcx, px)
                    # evict PSUM (frees the accumulator), then normalize from
                    # SBUF: r = 1/rowsum broadcast on gpsimd.  NOTE: the
                    # reciprocal's input must be a partition-0-based tile —
                    # reciprocal_approx_fast on a partition-offset view
                    # produces garbage on HW (sim is fine).
                    oo = bs.tile([64, 2, 512], F32, tag="oo", bufs=2, name="oo")
                    nc.vector.tensor_copy(oo[:], oab[0:64])
                    ssum = bs.tile([1, 2, 512], F32, tag="ssum", bufs=2, name="ssum")
                    nc.vector.tensor_copy(ssum[:], oab[64:65, :, :])
                    rrf = bs.tile([1, 2, 512], F32, tag="rrf", bufs=2, name="rrf")
                    nc.vector.reciprocal_approx_fast(rrf[:], ssum[:])
                    rrb = bs.tile([1, 2, 512], BF16, tag="rrb", bufs=2, name="rrb")
                    nc.vector.tensor_copy(rrb[:], rrf[:])
                    rbs = bs.tile([64, 2, 512], BF16, tag="rbs", bufs=2, name="rbs")
                    nc.gpsimd.partition_broadcast(rbs[:], rrb[:], channels=64)
                    nc.vector.tensor_mul(
                        oT[0:64, m, 512 * j:512 * (j + 1)], oo[0:64, 0, :], rbs[:, 0, :]
                    )
                    nc.vector.tensor_mul(
                        oT[64:128, m, 512 * j:512 * (j + 1)], oo[0:64, 1, :], rbs[:, 1, :]
                    )

                # ---------- emission schedule ----------
                kunit(0)
                vunit(0); vunit(1)
                qunit(0, 0)
                krope(0)
                vunit(2); vunit(3)
                qrope(0, 0)
                qunit(0, 1); qrope(0, 1)
                qunit(0, 2); qrope(0, 2)
                qunit(0, 3); qrope(0, 3)

                # background queue: work for iteration j+1 (projections,
                # rope) and the out-proj of j-1, drained one unit per
                # attention kc-pair; leftovers flushed at each j boundary.
                def push_bg(j):
                    if j < NQT - 1:
                        bg.append(lambda: kunit(j + 1))
                        bg.append(lambda: krope(j + 1))
                        bg.append(lambda: qunit(j + 1, 0))
                        bg.append(lambda: qrope(j + 1, 0))
                    if j >= 1:
                        bg.append(lambda: load_og(j - 1))
                        for tt in range(4):
                            bg.append(lambda tt=tt: emit_po(j - 1, tt))
                    if j < NQT - 1:
                        for m in range(1, 4):
                            bg.append(lambda m=m: qunit(j + 1, m))
                            bg.append(lambda m=m: qrope(j + 1, m))
                        for t in range(4):
                            bg.append(lambda t=t: vunit(4 * (j + 1) + t))

                for j in range(NQT):
                    while bg:  # leftovers are needed by this iteration
                        bg.pop(0)()
                    push_bg(j)
                    for m in range(4):
                        attention(j, m)
                    emit_rs(j)
                # tail: own-half out-proj covers the last RS's flight time
                emit_po_own(3, 0)
                emit_po_own(3, 1)
                load_og(3)
                emit_po_peer(3, 0)
                emit_po_peer(3, 1)
                emit_po_own(3, 2)
                emit_po_peer(3, 2)
                emit_po_own(3, 3)
                emit_po_peer(3, 3)

    nc.finalize()
    return nc


_NC_CACHE = None


def _get_nc():
    global _NC_CACHE
    if _NC_CACHE is None:
        _NC_CACHE = build()
    return _NC_CACHE


def _make_in_maps(inputs):
    x = np.asarray(inputs["x"], dtype=np.float32)
    wq = np.asarray(inputs["wq"], dtype=np.float32)
    wk = np.asarray(inputs["wk"], dtype=np.float32)
    wv = np.asarray(inputs["wv"], dtype=np.float32)
    wo = np.asarray(inputs["wo"], dtype=np.float32)
    bf = ml_dtypes.bfloat16
    in_maps = []
    for i in range(8):
        b, p = i // 2, i % 2
        # wo rows permuted to [own-head rows; peer-head rows]
        wo_perm = np.concatenate(
            [wo[p * DQ:(p + 1) * DQ], wo[(1 - p) * DQ:(2 - p) * DQ]], axis=0
        )[:, p * DQ:(p + 1) * DQ]
        msk = np.ones((128, 2), np.float32)
        msk[:, p] = 0.0
        in_maps.append({
            "x": np.ascontiguousarray(x[b].T.astype(bf)),  # pre-transposed [C, N]
            "wq": np.ascontiguousarray(wq[:, p * DQ:(p + 1) * DQ].astype(bf)),
            "wk": np.ascontiguousarray(wk[:, p * DKV:(p + 1) * DKV].astype(bf)),
            "wv": np.ascontiguousarray(wv[:, p * DKV:(p + 1) * DKV].astype(bf)),
            "wo": np.ascontiguousarray(wo_perm.astype(bf)),
            "msk": np.ascontiguousarray(msk),
        })
    return in_maps


def kernel(x, wq, wk, wv, wo):
    x = np.asarray(x, dtype=np.float32)
    B = x.shape[0]
    nc = _get_nc()
    in_maps = _make_in_maps({"x": x, "wq": wq, "wk": wk, "wv": wv, "wo": wo})
    res = run_bass_kernel_spmd(nc, in_maps, core_ids=list(range(8)))
    out = np.empty((B, N, C), dtype=np.float32)
    for b in range(B):
        out[b, :, 0:DQ] = res.results[2 * b]["out"]
        out[b, :, DQ:C] = res.results[2 * b + 1]["out"]
    return out
